# revision 1
# baseline (speedup 1.0000x reference)
"""DarkChannelPrior kernel for 8 Trainium2 NeuronCores.

Computes: dark = min over 3 channels of images [16,3,1024,1024], then a
15x15 box-average convolution (padding 7) -> [16,1,1024,1024].

Strategy:
  - Pure data parallel: 2 images per core across 8 cores.
  - Channel min on VectorE (2 tensor_tensor(min) ops per tile), output bf16.
  - The separable box filter runs as two banded-matmul passes on TensorE.
    Each pass computes out[a, b] = sum_{|b'-b|<=7} in[b', a] -- i.e. a
    15-tap sum along the partition axis fused with a transpose (the input
    tile is the stationary operand, the 0/1 band matrix is the moving
    operand).  Two passes restore the original orientation and give the
    full 2D box sum.  Input rows are tiled 128-wide at stride 112 so each
    output tile needs exactly one source tile (8-row halo overlap, keeping
    every slice offset 4-byte aligned).
  - ScalarE copies PSUM->SBUF; the final copy applies the 1/225 scale.
"""

import numpy as np
import ml_dtypes

import concourse.bacc as bacc
import concourse.bass as bass
import concourse.tile as tile
import concourse.mybir as mybir
from concourse.bass_utils import run_bass_kernel_spmd

KS = 15
PAD = KS // 2
H = W = 1024
IMGS_PER_CORE = 2
N_CORES = 8
# Stride 112 (not the minimal 114) keeps every tile start EVEN, so all bf16
# lhsT free-dim slices are 4-byte aligned (PE fast-weight-load reads 32 bits
# at a time).  Interior tiles carry an 8-row halo above and 8 below.
STRIDE = 112
NT = 10

# Row-tile t holds image rows [T_START[t], T_START[t]+T_LEN[t]) in
# partitions [0, T_LEN[t]).
T_START = [0] + [STRIDE * t - (PAD + 1) for t in range(1, NT)]
T_LEN = [120] + [128] * (NT - 2) + [1024 - (STRIDE * (NT - 1) - (PAD + 1))]
# Output rows produced from tile t: [STRIDE*t, STRIDE*t + OUT_W[t])
OUT_W = [STRIDE] * (NT - 1) + [1024 - STRIDE * (NT - 1)]  # 112*9, 16

LAST_RESULTS = None
_PROGRAM_CACHE = {}

# Emission order for the load+min phase: t-major interleaves both images'
# DMA streams; i-major finishes image 0 first so its passes start earlier.
_LOAD_ORDER = [(t, i) for i in range(IMGS_PER_CORE) for t in range(NT)]


def _build_bmat():
    """Band matrices as one [128, 224] bf16 tensor.

    cols 0:112   B_mid:   B[k, j] = 1 iff 1 <= k-j <= 15   (tiles with halo:
                 partition k is image row STRIDE*t - 8 + k)
    cols 112:224 B_first: B[k, j] = 1 iff |k-j| <= 7        (first tile, no
                 top halo: partition k is image row k)
    """
    B = np.zeros((128, 2 * STRIDE), dtype=np.float32)
    k = np.arange(128)[:, None]
    j = np.arange(STRIDE)[None, :]
    B[:, 0:STRIDE] = ((k - j >= 1) & (k - j <= 2 * PAD + 1)).astype(np.float32)
    k2 = np.arange(T_LEN[0])[:, None]
    B[0 : T_LEN[0], STRIDE : 2 * STRIDE] = (np.abs(k2 - j) <= PAD).astype(
        np.float32
    )
    return B.astype(ml_dtypes.bfloat16)


def _jslices(s):
    """Output-column pieces for source tile s, split at the PSUM bank
    boundary (512).  Yields (out_col, b_col, width)."""
    o0 = STRIDE * s
    w = OUT_W[s]
    if o0 < 512 < o0 + w:
        cut = 512 - o0
        return [(o0, 0, cut), (512, cut, w - cut)]
    return [(o0, 0, w)]


def _boxsum_pass(nc, pspool, src, dst_writer, bmat):
    """One fused pass: for each 128-wide output block of the transposed
    orientation, matmul-accumulate the 15-tap partition sums from the 9
    overlapped source tiles of `src`, then hand both [128,512] PSUM banks
    to dst_writer(bank_idx, psum_ap, n_valid_partitions, blk).

    src(blk, s, n) -> lhsT AP [T_LEN[s], n] for source tile s, output block blk.
    """
    for blk in range(src.nblocks):
        ncols = src.blk_len[blk]
        psA = pspool.tile([128, 512], mybir.dt.float32, tag="ps")
        psB = pspool.tile([128, 512], mybir.dt.float32, tag="ps")
        for s in range(NT):
            nr = T_LEN[s]
            for (po, bo, w) in _jslices(s):
                if s == 0:
                    rhs = bmat[0:nr, STRIDE + bo : STRIDE + bo + w]
                else:
                    rhs = bmat[0:nr, bo : bo + w]
                ps, off = (psA, po) if po < 512 else (psB, po - 512)
                nc.tensor.matmul(
                    ps[0:ncols, off : off + w],
                    lhsT=src.lhsT(blk, s, ncols),
                    rhs=rhs,
                    start=True,
                    stop=True,
                )
        dst_writer(0, psA, ncols, blk)
        dst_writer(1, psB, ncols, blk)


class _Pass1Src:
    """Pass-1 source: dark-channel tiles.  Output blocks are the overlapped
    c-blocks (so pass-2 can contract over them without partition shifts)."""

    def __init__(self, dov):
        self.dov = dov
        self.nblocks = NT
        self.blk_len = T_LEN

    def lhsT(self, blk, s, ncols):
        c0 = T_START[blk]
        return self.dov[0 : T_LEN[s], s * W + c0 : s * W + c0 + ncols]


class _Pass2Src:
    """Pass-2 source: pass-1 output tiles [c, r].  Output blocks are the 8
    aligned 128-row blocks of the final image."""

    def __init__(self, t1):
        self.t1 = t1
        self.nblocks = 8
        self.blk_len = [128] * 8

    def lhsT(self, blk, s, ncols):
        return self.t1[0 : T_LEN[s], s * H + blk * 128 : s * H + blk * 128 + ncols]


def _build_program(scale, reps=1, mode="full"):
    # Bacc (not raw Bass): its compile() pipeline splits multi-wait
    # instructions via event semaphores, which TRN2 walrus codegen requires.
    # reps>1 wraps the body in a For_i loop (benchmarking only).
    nc = bacc.Bacc(
        "TRN2", target_bir_lowering=False, debug=False, num_devices=N_CORES
    )
    x = nc.dram_tensor(
        "x", [IMGS_PER_CORE, 3, H, W], mybir.dt.float32, kind="ExternalInput"
    )
    bm = nc.dram_tensor(
        "bmat", [128, 2 * STRIDE], mybir.dt.bfloat16, kind="ExternalInput"
    )
    y = nc.dram_tensor(
        "y", [IMGS_PER_CORE, H, W], mybir.dt.float32, kind="ExternalOutput"
    )

    with tile.TileContext(nc) as tc:
        with (
            tc.tile_pool(name="const", bufs=1) as cpool,
            tc.tile_pool(name="chan", bufs=4) as chpool,
            tc.tile_pool(name="dov", bufs=2) as dpool,
            tc.tile_pool(name="t1", bufs=2) as t1pool,
            tc.tile_pool(name="outp", bufs=6) as opool,
            tc.tile_pool(name="psum", bufs=8, space="PSUM") as pspool,
        ):
            bmat = cpool.tile([128, 2 * STRIDE], mybir.dt.bfloat16)
            nc.sync.dma_start(bmat[:], bm[:])

            import contextlib

            loop_cm = tc.For_i(0, reps, 1) if reps > 1 else contextlib.nullcontext()
            with loop_cm:
                _emit_images(
                    nc, tc, x, y, bmat, scale, chpool, dpool, t1pool, opool,
                    pspool, mode,
                )
    nc.compile()
    return nc


def _emit_images(
    nc, tc, x, y, bmat, scale, chpool, dpool, t1pool, opool, pspool, mode="full"
):
    do_dma = mode in ("full", "dma")
    do_compute = mode in ("full", "compute")

    # --- channel min into overlapped row tiles (bf16), both images
    # interleaved t-major so the input DMA stream never starves ---
    dovs = [
        dpool.tile([128, NT * W], mybir.dt.bfloat16, tag="dov", name=f"dov{i}")
        for i in range(IMGS_PER_CORE)
    ]
    csts = []
    if mode == "compute":
        for i in range(IMGS_PER_CORE):
            cst = chpool.tile(
                [128, 3 * W], mybir.dt.float32, tag="ch", name=f"cs{i}"
            )
            nc.sync.dma_start(cst[:], x[i, :, 0:128, :])
            csts.append(cst)
    for t, i in _LOAD_ORDER:
        if True:
            r0, nr = T_START[t], T_LEN[t]
            dov = dovs[i]
            if mode == "compute":
                ch = csts[i]
            else:
                ch = chpool.tile([128, 3 * W], mybir.dt.float32, tag="ch")
                for c in range(3):
                    nc.sync.dma_start(
                        ch[:nr, c * W : (c + 1) * W], x[i, c, r0 : r0 + nr, :]
                    )
                if mode == "dma" and t < 8:
                    nc.sync.dma_start(
                        y[i, t * 128 : t * 128 + nr, :], ch[:nr, 0:W]
                    )
            if do_compute:
                dst = dov[:nr, t * W : (t + 1) * W]
                if mode == "compute":
                    nc.vector.tensor_tensor(
                        dst, ch[:nr, 0:W], ch[:nr, W : 2 * W], mybir.AluOpType.min
                    )
                    nc.vector.tensor_tensor(
                        dst,
                        ch[:nr, 0:W],
                        ch[:nr, 2 * W : 3 * W],
                        mybir.AluOpType.min,
                    )
                else:
                    nc.vector.tensor_tensor(
                        ch[:nr, 0:W],
                        ch[:nr, 0:W],
                        ch[:nr, W : 2 * W],
                        mybir.AluOpType.min,
                    )
                    nc.vector.tensor_tensor(
                        dst,
                        ch[:nr, 0:W],
                        ch[:nr, 2 * W : 3 * W],
                        mybir.AluOpType.min,
                    )

    if do_compute:
        for i in range(IMGS_PER_CORE):
            # --- pass 1: 15-tap row sums, transposed layout [c, r] ---
            t1 = t1pool.tile([128, NT * H], mybir.dt.bfloat16, tag="t1")

            def write_t1(bank, ps, ncols, blk, t1=t1):
                nc.scalar.activation(
                    t1[0:ncols, blk * H + bank * 512 : blk * H + bank * 512 + 512],
                    ps[0:ncols, :],
                    mybir.ActivationFunctionType.Copy,
                )

            _boxsum_pass(nc, pspool, _Pass1Src(dovs[i]), write_t1, bmat)

            # --- pass 2: 15-tap col sums, back to [r, c]; scale + store ---
            _boxsum_pass(
                nc,
                pspool,
                _Pass2Src(t1),
                _make_out_writer(nc, opool, y, i, scale, do_dma),
                bmat,
            )
def _make_out_writer(nc, opool, y, i, scale, do_dma=True):
    state = {}

    def write_out(bank, ps, ncols, blk):
        if blk not in state:
            state[blk] = opool.tile(
                [128, W], mybir.dt.float32, tag="out", name=f"out_{i}_{blk}"
            )
        ot = state[blk]
        nc.scalar.activation(
            ot[:, bank * 512 : bank * 512 + 512],
            ps[:, :],
            mybir.ActivationFunctionType.Copy,
            scale=scale,
        )
        if bank == 1:
            if do_dma:
                nc.sync.dma_start(y[i, blk * 128 : (blk + 1) * 128, :], ot[:])
            del state[blk]

    return write_out


def kernel(images, weight):
    global LAST_RESULTS
    images = np.ascontiguousarray(np.asarray(images, dtype=np.float32))
    weight = np.asarray(weight, dtype=np.float64)
    # reference: conv with w = weight/225; weight is uniform (ones), so the
    # whole filter reduces to mean(weight)/225 * boxsum.
    scale = float(weight.mean()) / (KS * KS)

    if scale not in _PROGRAM_CACHE:
        _PROGRAM_CACHE[scale] = _build_program(scale)
    nc = _PROGRAM_CACHE[scale]
    bmat = _build_bmat()
    in_maps = [
        {
            "x": images[c * IMGS_PER_CORE : (c + 1) * IMGS_PER_CORE],
            "bmat": bmat,
        }
        for c in range(N_CORES)
    ]
    res = run_bass_kernel_spmd(nc, in_maps, core_ids=list(range(N_CORES)))
    LAST_RESULTS = res
    out = np.concatenate([r["y"][:, None, :, :] for r in res.results], axis=0)
    return out.astype(np.float32)



# revision 3
# speedup vs baseline: 354.1589x; 354.1589x over previous
"""DarkChannelPrior kernel for 8 Trainium2 NeuronCores.

Computes: dark = min over 3 channels of images [16,3,1024,1024], then a
15x15 box-average convolution (padding 7) -> [16,1,1024,1024].

Strategy (streaming slab pipeline, pure data parallel, 2 images/core):
  - The image is cut into 9 row slabs of 114 output rows (114*8+112=1024).
    Slab j loads input rows [114j-7, 114j+121) (<=128, so one SBUF
    partition tile with the 7-row conv halo included).
  - Per slab: DMA 3 channel tiles -> VectorE channel-min (fp32 then ->bf16)
    -> TensorE banded matmul for the 15-tap ROW sums (the 0/1 band matrix
    is the stationary operand [rows, 114], the dark slab is the moving
    operand, so the output stays in [row, col] orientation) -> ScalarE
    copies PSUM->SBUF with the 1/225 scale fused -> VectorE computes the
    15-tap COLUMN sums in a single pass with tensor_tensor_scan
    (state = (t1[v] + state) - t1[v-15], i.e. a sliding-window recurrence
    over a zero-padded buffer) -> DMA the 114 finished rows out.
  - Every slab is independent, so DMA/PE/DVE/Act all overlap; the kernel
    is bound by the input DMA stream.
"""

import numpy as np
import ml_dtypes

import concourse.bacc as bacc
import concourse.bass as bass
import concourse.tile as tile
import concourse.mybir as mybir
from concourse.bass_utils import run_bass_kernel_spmd

KS = 15
PAD = KS // 2
H = W = 1024
IMGS_PER_CORE = 2
N_CORES = 8

STRIDE = 114  # output rows per slab; 114+2*PAD = 128 = full partition dim
NSLAB = 9
SLAB_R0 = [STRIDE * j for j in range(NSLAB)]                     # first out row
SLAB_NJ = [STRIDE] * (NSLAB - 1) + [H - STRIDE * (NSLAB - 1)]    # out rows (114/112)
SLAB_IN0 = [max(0, STRIDE * j - PAD) for j in range(NSLAB)]      # first in row
SLAB_NR = [
    min(H, SLAB_R0[j] + SLAB_NJ[j] + PAD) - SLAB_IN0[j] for j in range(NSLAB)
]  # in rows: 121, 7x128, 119

# scan geometry: window sum W[v] = sum_{u in (v-15, v]} t1[u], t1 zero-padded.
# padbuf cols [PB_LO, PB_LO+W) hold t1; data0 = [PB_LO:PB_LO+NV),
# data1 = [0:NV), out col v maps to image col v-PAD (keep v in [PAD, PAD+W)).
PB_LO = KS          # 15 zeros in front (data1 reads t1[v-15])
NV = W + PAD        # scan positions
PB_W = KS + W + PAD  # 1046

LAST_RESULTS = None
_PROGRAM_CACHE = {}


def _build_bmat():
    """Band matrices as one [128, 228] bf16 tensor.

    cols 0:114    B_mid:   B[k, m] = 1 iff 0 <= k-m <= 14  (slabs with a top
                  halo: partition k is image row SLAB_IN0[j] + k)
    cols 114:228  B_first: B[k, m] = 1 iff |k-m| <= 7       (slab 0, no top
                  halo)
    """
    B = np.zeros((128, 2 * STRIDE), dtype=np.float32)
    k = np.arange(128)[:, None]
    m = np.arange(STRIDE)[None, :]
    B[:, 0:STRIDE] = ((k - m >= 0) & (k - m <= 2 * PAD)).astype(np.float32)
    B[:, STRIDE : 2 * STRIDE] = (np.abs(k - m) <= PAD).astype(np.float32)
    return B.astype(ml_dtypes.bfloat16)


def _build_program(scale, reps=1):
    # Bacc (not raw Bass): its compile() pipeline splits multi-wait
    # instructions via event semaphores, which TRN2 walrus codegen requires.
    # reps>1 wraps the body in a For_i loop (benchmarking only).
    nc = bacc.Bacc(
        "TRN2", target_bir_lowering=False, debug=False, num_devices=N_CORES
    )
    x = nc.dram_tensor(
        "x", [IMGS_PER_CORE, 3, H, W], mybir.dt.float32, kind="ExternalInput"
    )
    bm = nc.dram_tensor(
        "bmat", [128, 2 * STRIDE], mybir.dt.bfloat16, kind="ExternalInput"
    )
    y = nc.dram_tensor(
        "y", [IMGS_PER_CORE, H, W], mybir.dt.float32, kind="ExternalOutput"
    )

    with tile.TileContext(nc) as tc:
        with (
            tc.tile_pool(name="const", bufs=1) as cpool,
            tc.tile_pool(name="chan", bufs=4) as chpool,
            tc.tile_pool(name="mintmp", bufs=2) as mpool,
            tc.tile_pool(name="dark", bufs=3) as dpool,
            tc.tile_pool(name="pad", bufs=1) as ppool,
            tc.tile_pool(name="outp", bufs=4) as opool,
            tc.tile_pool(name="psum", bufs=8, space="PSUM") as pspool,
        ):
            bmat = cpool.tile([128, 2 * STRIDE], mybir.dt.bfloat16)
            nc.sync.dma_start(bmat[:], bm[:])

            # pad buffers live across the whole program so their zero edges
            # are set once, outside the timing loop
            padbufs = []
            for i in range(IMGS_PER_CORE):
                for j in range(NSLAB):
                    pb = ppool.tile(
                        [128, PB_W], mybir.dt.float32, name=f"pad_{i}_{j}"
                    )
                    nc.vector.memset(pb[:, 0:PB_LO], 0.0)
                    nc.vector.memset(pb[:, PB_LO + W : PB_W], 0.0)
                    padbufs.append(pb)

            import contextlib

            loop_cm = tc.For_i(0, reps, 1) if reps > 1 else contextlib.nullcontext()
            with loop_cm:
                for i in range(IMGS_PER_CORE):
                    for j in range(NSLAB):
                        _emit_slab(
                            nc, x, y, bmat, scale, i, j,
                            padbufs[i * NSLAB + j],
                            chpool, mpool, dpool, opool, pspool,
                        )
    nc.compile()
    return nc


def _emit_slab(nc, x, y, bmat, scale, i, j, pb, chpool, mpool, dpool, opool,
               pspool):
    r0, nj = SLAB_R0[j], SLAB_NJ[j]
    in0, nr = SLAB_IN0[j], SLAB_NR[j]

    # --- load 3 channel slabs ---
    ch = chpool.tile([128, 3 * W], mybir.dt.float32, tag="ch")
    for c in range(3):
        nc.sync.dma_start(ch[:nr, c * W : (c + 1) * W], x[i, c, in0 : in0 + nr, :])

    # --- channel min (fp32 x fp32 -> fp32, then -> bf16) ---
    mt = mpool.tile([128, W], mybir.dt.float32, tag="mt")
    nc.vector.tensor_tensor(
        mt[:nr, :], ch[:nr, 0:W], ch[:nr, W : 2 * W], mybir.AluOpType.min
    )
    dark = dpool.tile([128, W], mybir.dt.bfloat16, tag="dark")
    nc.vector.tensor_tensor(
        dark[:nr, :], mt[:nr, :], ch[:nr, 2 * W : 3 * W], mybir.AluOpType.min
    )

    # --- 15-tap row sums on TensorE: band stationary, dark moving ---
    if j == 0:
        lhsT = bmat[0:nr, STRIDE : STRIDE + nj]
    else:
        lhsT = bmat[0:nr, 0:nj]
    for half in range(2):
        ps = pspool.tile([128, 512], mybir.dt.float32, tag="ps")
        nc.tensor.matmul(
            ps[0:nj, :],
            lhsT=lhsT,
            rhs=dark[0:nr, half * 512 : (half + 1) * 512],
            start=True,
            stop=True,
        )
        # --- PSUM -> padbuf with 1/225 scale fused ---
        nc.scalar.activation(
            pb[0:nj, PB_LO + half * 512 : PB_LO + (half + 1) * 512],
            ps[0:nj, :],
            mybir.ActivationFunctionType.Copy,
            scale=scale,
        )

    # --- 15-tap column sums in one DVE pass: sliding-window recurrence ---
    ot = opool.tile([128, NV], mybir.dt.float32, tag="out")
    nc.vector.tensor_tensor_scan(
        ot[0:nj, :],
        pb[0:nj, PB_LO : PB_LO + NV],
        pb[0:nj, 0:NV],
        0.0,
        mybir.AluOpType.add,
        mybir.AluOpType.subtract,
    )

    # --- store the finished rows (skip the PAD warmup cols) ---
    nc.sync.dma_start(y[i, r0 : r0 + nj, :], ot[0:nj, PAD : PAD + W])


def kernel(images, weight):
    global LAST_RESULTS
    images = np.ascontiguousarray(np.asarray(images, dtype=np.float32))
    weight = np.asarray(weight, dtype=np.float64)
    # reference: conv with w = weight/225; weight is uniform (ones), so the
    # whole filter reduces to mean(weight)/225 * boxsum.
    scale = float(weight.mean()) / (KS * KS)

    if scale not in _PROGRAM_CACHE:
        _PROGRAM_CACHE[scale] = _build_program(scale)
    nc = _PROGRAM_CACHE[scale]
    bmat = _build_bmat()
    in_maps = [
        {
            "x": images[c * IMGS_PER_CORE : (c + 1) * IMGS_PER_CORE],
            "bmat": bmat,
        }
        for c in range(N_CORES)
    ]
    res = run_bass_kernel_spmd(nc, in_maps, core_ids=list(range(N_CORES)))
    LAST_RESULTS = res
    out = np.concatenate([r["y"][:, None, :, :] for r in res.results], axis=0)
    return out.astype(np.float32)


# revision 22
# speedup vs baseline: 507.6287x; 1.4333x over previous
"""DarkChannelPrior kernel for 8 Trainium2 NeuronCores.

Computes: dark = min over 3 channels of images [16,3,1024,1024], then a
15x15 box-average convolution (padding 7) -> [16,1,1024,1024].

Strategy (streaming slab pipeline, pure data parallel, 2 images/core):
  - The image is cut into 8 flat row slabs of 128 rows (no halo in the
    DMA: every input byte is read exactly once, 25.2 MB/core).
  - Per slab: one 3-channel DMA (SP HWDGE ring) -> VectorE channel-min
    (fp32 then ->bf16) into a per-image dark buffer [128, 8*1024]
    -> TensorE computes the 15-tap ROW sums with up to 3 accumulating
    banded matmuls (band stationary [rows, 128], dark slab moving): the
    main band covers this slab's 128 rows and two 7-row bands pull the
    conv halo from the neighbouring slabs' dark columns, so the output
    stays in [row, col] orientation -> ScalarE copies PSUM->SBUF with the
    1/225 scale fused -> VectorE computes the 15-tap COLUMN sums in a
    single pass with tensor_tensor_scan (state = (t1[v] + state) -
    t1[v-15], a sliding-window recurrence over a zero-padded buffer)
    -> DMA the 128 finished rows out on the Activation HWDGE ring.
  - Every stage is per-slab so DMA/PE/DVE/Act overlap; the kernel is
    bound by the HBM read stream + write stream.
"""

import numpy as np
import ml_dtypes

import concourse.bacc as bacc
import concourse.bass as bass
import concourse.tile as tile
import concourse.mybir as mybir
from concourse.bass_utils import run_bass_kernel_spmd

KS = 15
PAD = KS // 2
H = W = 1024
IMGS_PER_CORE = 2
N_CORES = 8

SLAB = 128
NSLAB = H // SLAB  # 8

# scan geometry: window sum S[v] = sum_{u in (v-15, v]} t1[u], t1 zero-padded.
# padbuf cols [PB_LO, PB_LO+W) hold t1; data0 = [PB_LO:PB_LO+NV),
# data1 = [0:NV), out col v maps to image col v-PAD (keep v in [PAD, PAD+W)).
PB_LO = KS          # 15 zeros in front (data1 reads t1[v-15])
NV = W + PAD        # scan positions
PB_W = KS + W + PAD  # 1046

LAST_RESULTS = None
_PROGRAM_CACHE = {}


def _build_bmat():
    """Band matrices as one [128, 384] bf16 tensor (k = partition).

    cols   0:128  B_main: B[k, m] = 1 iff |k-m| <= 7    (this slab's rows)
    cols 128:256  B_prev: B[k, m] = 1 iff 121 <= k-m <= 127 (prev slab's
                  rows read as a base-64 64-deep contraction: partition k
                  is prev-slab image row 128*(t-1)+k, nonzero only k>=121)
    cols 256:384  B_next: B[k, m] = 1 iff 121+k <= m     (next slab's first 7
                  rows), k in [0, 7)
    """
    B = np.zeros((128, 3 * SLAB), dtype=np.float32)
    k = np.arange(128)[:, None]
    m = np.arange(SLAB)[None, :]
    B[:, 0:SLAB] = (np.abs(k - m) <= PAD).astype(np.float32)
    B[:, SLAB : 2 * SLAB] = ((k - m >= 121) & (k - m <= 127)).astype(np.float32)
    B[:, 2 * SLAB : 3 * SLAB] = ((k <= 2 * PAD - 1) & (m >= 121 + k)).astype(
        np.float32
    )
    return B.astype(ml_dtypes.bfloat16)


def _build_program(scale, reps=1, mode="full"):
    # Bacc (not raw Bass): its compile() pipeline splits multi-wait
    # instructions via event semaphores, which TRN2 walrus codegen requires.
    # reps>1 wraps the body in a For_i loop (benchmarking only).
    nc = bacc.Bacc(
        "TRN2", target_bir_lowering=False, debug=False, num_devices=N_CORES
    )
    x = nc.dram_tensor(
        "x", [IMGS_PER_CORE, 3, H, W], mybir.dt.float32, kind="ExternalInput"
    )
    bm = nc.dram_tensor(
        "bmat", [128, 3 * SLAB], mybir.dt.bfloat16, kind="ExternalInput"
    )
    y = nc.dram_tensor(
        "y", [IMGS_PER_CORE, H, W], mybir.dt.float32, kind="ExternalOutput"
    )

    with tile.TileContext(nc) as tc:
        with (
            tc.tile_pool(name="const", bufs=1) as cpool,
            tc.tile_pool(name="chan", bufs=4) as chpool,
            tc.tile_pool(name="mintmp", bufs=2) as mpool,
            tc.tile_pool(name="darkp", bufs=1) as dpool,
            tc.tile_pool(name="pad", bufs=1) as ppool,
            tc.tile_pool(name="outp", bufs=5) as opool,
            tc.tile_pool(name="psum", bufs=8, space="PSUM") as pspool,
        ):
            bmat = cpool.tile([128, 3 * SLAB], mybir.dt.bfloat16)
            nc.sync.dma_start(bmat[:], bm[:])

            # per-image dark buffers and pad buffers live across the whole
            # program: dark so neighbouring slabs can read each other's
            # halo columns, pad so the zero edges are set once
            darks = [
                dpool.tile([128, NSLAB * W], mybir.dt.bfloat16, name=f"dark{i}")
                for i in range(IMGS_PER_CORE)
            ]
            padbufs = []
            for i in range(IMGS_PER_CORE):
                for t in range(NSLAB):
                    pb = ppool.tile(
                        [128, PB_W], mybir.dt.float32, name=f"pad_{i}_{t}"
                    )
                    nc.vector.memset(pb[:, 0:PB_LO], 0.0)
                    nc.vector.memset(pb[:, PB_LO + W : PB_W], 0.0)
                    padbufs.append(pb)

            import contextlib

            loop_cm = tc.For_i(0, reps, 1) if reps > 1 else contextlib.nullcontext()
            with loop_cm:
                if mode == "noop":
                    nt = mpool.tile([128, W], mybir.dt.float32, tag="mt")
                    nc.vector.memset(nt[:, 0:8], 0.0)
                else:
                    for i in range(IMGS_PER_CORE):
                        for t in range(NSLAB):
                            _emit_load_min(nc, x, y, darks[i], i, t, chpool,
                                           mpool, mode)
                            if mode != "full":
                                continue
                            if t >= 1:
                                _emit_boxfilter(
                                    nc, y, bmat, scale, darks[i], i, t - 1,
                                    padbufs[i * NSLAB + t - 1], opool, pspool,
                                )
                        if mode == "full":
                            _emit_boxfilter(
                                nc, y, bmat, scale, darks[i], i, NSLAB - 1,
                                padbufs[i * NSLAB + NSLAB - 1], opool, pspool,
                            )
    nc.compile()
    return nc


def _emit_load_min(nc, x, y, dark, i, t, chpool, mpool, mode):
    r0 = t * SLAB
    ch = chpool.tile([128, 3 * W], mybir.dt.float32, tag="ch")
    if mode in ("full", "dma", "dmain"):
        # one DMA for all 3 channels: src [128, 3, W] via AP transpose
        nc.sync.dma_start(
            ch[:, :], x[i, :, r0 : r0 + SLAB, :].transpose([1, 0, 2])
        )
    if mode == "dma":
        nc.scalar.dma_start(y[i, r0 : r0 + SLAB, :], ch[0:SLAB, 0:W])
    if mode != "full":
        return
    mt = mpool.tile([128, W], mybir.dt.float32, tag="mt")
    nc.vector.tensor_tensor(
        mt[:, :], ch[:, 0:W], ch[:, W : 2 * W], mybir.AluOpType.min
    )
    nc.vector.tensor_tensor(
        dark[:, t * W : (t + 1) * W], mt[:, :], ch[:, 2 * W : 3 * W],
        mybir.AluOpType.min,
    )


def _emit_boxfilter(nc, y, bmat, scale, dark, i, t, pb, opool, pspool):
    r0 = t * SLAB

    # --- 15-tap row sums on TensorE: bands stationary, dark moving; the
    # halo rows come from the neighbour slabs' dark columns via two extra
    # accumulating matmuls ---
    for half in range(2):
        c0 = half * 512
        ps = pspool.tile([128, 512], mybir.dt.float32, tag="ps")
        mms = [(bmat[0:128, 0:SLAB], dark[0:128, t * W + c0 : t * W + c0 + 512])]
        if t > 0:
            mms.append((
                bmat[64:128, SLAB : 2 * SLAB],
                dark[64:128, (t - 1) * W + c0 : (t - 1) * W + c0 + 512],
            ))
        if t < NSLAB - 1:
            mms.append((
                bmat[0:PAD, 2 * SLAB : 3 * SLAB],
                dark[0:PAD, (t + 1) * W + c0 : (t + 1) * W + c0 + 512],
            ))
        for k, (lhsT, rhs) in enumerate(mms):
            nc.tensor.matmul(
                ps[:, :],
                lhsT=lhsT,
                rhs=rhs,
                start=(k == 0),
                stop=(k == len(mms) - 1),
            )
        # --- PSUM -> padbuf with 1/225 scale fused ---
        nc.scalar.activation(
            pb[:, PB_LO + c0 : PB_LO + c0 + 512],
            ps[:, :],
            mybir.ActivationFunctionType.Copy,
            scale=scale,
        )

    # --- 15-tap column sums in one DVE pass: sliding-window recurrence ---
    ot = opool.tile([128, NV], mybir.dt.float32, tag="out")
    nc.vector.tensor_tensor_scan(
        ot[:, :],
        pb[:, PB_LO : PB_LO + NV],
        pb[:, 0:NV],
        0.0,
        mybir.AluOpType.add,
        mybir.AluOpType.subtract,
    )

    # --- store the finished rows (skip the PAD warmup cols); issued from
    # the Activation engine -> second HWDGE ring so the output stream does
    # not serialize behind the input stream ---
    nc.scalar.dma_start(y[i, r0 : r0 + SLAB, :], ot[:, PAD : PAD + W])


def kernel(images, weight):
    global LAST_RESULTS
    images = np.ascontiguousarray(np.asarray(images, dtype=np.float32))
    weight = np.asarray(weight, dtype=np.float64)
    # reference: conv with w = weight/225; weight is uniform (ones), so the
    # whole filter reduces to mean(weight)/225 * boxsum.
    scale = float(weight.mean()) / (KS * KS)

    if scale not in _PROGRAM_CACHE:
        _PROGRAM_CACHE[scale] = _build_program(scale)
    nc = _PROGRAM_CACHE[scale]
    bmat = _build_bmat()
    in_maps = [
        {
            "x": images[c * IMGS_PER_CORE : (c + 1) * IMGS_PER_CORE],
            "bmat": bmat,
        }
        for c in range(N_CORES)
    ]
    res = run_bass_kernel_spmd(nc, in_maps, core_ids=list(range(N_CORES)))
    LAST_RESULTS = res
    out = np.concatenate([r["y"][:, None, :, :] for r in res.results], axis=0)
    return out.astype(np.float32)


# revision 25
# speedup vs baseline: 556.6415x; 1.0966x over previous
"""DarkChannelPrior kernel for 8 Trainium2 NeuronCores.

Computes: dark = min over 3 channels of images [16,3,1024,1024], then a
15x15 box-average convolution (padding 7) -> [16,1,1024,1024].

Strategy (streaming slab pipeline, pure data parallel, 2 images/core):
  - The image is cut into 8 flat row slabs of 128 rows (no halo in the
    DMA: every input byte is read exactly once, 25.2 MB/core).
  - Per slab: one 3-channel DMA (SP HWDGE ring) -> VectorE channel-min
    (fp32 then ->bf16) into a per-image dark buffer [128, 8*1024]
    -> TensorE computes the 15-tap ROW sums with up to 3 accumulating
    banded matmuls (band stationary [rows, 128], dark slab moving): the
    main band covers this slab's 128 rows and two 7-row bands pull the
    conv halo from the neighbouring slabs' dark columns, so the output
    stays in [row, col] orientation -> ScalarE copies PSUM->SBUF with the
    1/225 scale fused -> VectorE computes the 15-tap COLUMN sums in a
    single pass with tensor_tensor_scan (state = (t1[v] + state) -
    t1[v-15], a sliding-window recurrence over a zero-padded buffer)
    -> DMA the 128 finished rows out on the Activation HWDGE ring.
  - Every stage is per-slab so DMA/PE/DVE/Act overlap; the kernel is
    bound by the HBM read stream + write stream.
"""

import numpy as np
import ml_dtypes

import concourse.bacc as bacc
import concourse.bass as bass
import concourse.tile as tile
import concourse.mybir as mybir
from concourse.bass_utils import run_bass_kernel_spmd

KS = 15
PAD = KS // 2
H = W = 1024
IMGS_PER_CORE = 2
N_CORES = 8

SLAB = 128
NSLAB = H // SLAB  # 8

# scan geometry: window sum S[v] = sum_{u in (v-15, v]} t1[u], t1 zero-padded.
# padbuf cols [PB_LO, PB_LO+W) hold t1; data0 = [PB_LO:PB_LO+NV),
# data1 = [0:NV), out col v maps to image col v-PAD (keep v in [PAD, PAD+W)).
PB_LO = KS          # 15 zeros in front (data1 reads t1[v-15])
NV = W + PAD        # scan positions
PB_W = KS + W + PAD  # 1046

LAST_RESULTS = None
_PROGRAM_CACHE = {}


def _build_bmat():
    """Band matrices as one [128, 384] bf16 tensor (k = partition).

    cols   0:128  B_main: B[k, m] = 1 iff |k-m| <= 7    (this slab's rows)
    cols 128:256  B_prev: B[k, m] = 1 iff 121 <= k-m <= 127 (prev slab's
                  rows read as a base-64 64-deep contraction: partition k
                  is prev-slab image row 128*(t-1)+k, nonzero only k>=121)
    cols 256:384  B_next: B[k, m] = 1 iff 121+k <= m     (next slab's first 7
                  rows), k in [0, 7)
    """
    B = np.zeros((128, 3 * SLAB), dtype=np.float32)
    k = np.arange(128)[:, None]
    m = np.arange(SLAB)[None, :]
    B[:, 0:SLAB] = (np.abs(k - m) <= PAD).astype(np.float32)
    B[:, SLAB : 2 * SLAB] = ((k - m >= 121) & (k - m <= 127)).astype(np.float32)
    B[:, 2 * SLAB : 3 * SLAB] = ((k <= 2 * PAD - 1) & (m >= 121 + k)).astype(
        np.float32
    )
    return B.astype(ml_dtypes.bfloat16)


def _build_program(scale, reps=1, mode="full"):
    # Bacc (not raw Bass): its compile() pipeline splits multi-wait
    # instructions via event semaphores, which TRN2 walrus codegen requires.
    # reps>1 wraps the body in a For_i loop (benchmarking only).
    nc = bacc.Bacc(
        "TRN2", target_bir_lowering=False, debug=False, num_devices=N_CORES
    )
    x = nc.dram_tensor(
        "x", [IMGS_PER_CORE, 3, H, W], mybir.dt.float32, kind="ExternalInput"
    )
    bm = nc.dram_tensor(
        "bmat", [128, 3 * SLAB], mybir.dt.bfloat16, kind="ExternalInput"
    )
    y = nc.dram_tensor(
        "y", [IMGS_PER_CORE, H, W], mybir.dt.float32, kind="ExternalOutput"
    )

    with tile.TileContext(nc) as tc:
        with (
            tc.tile_pool(name="const", bufs=1) as cpool,
            tc.tile_pool(name="chan", bufs=4) as chpool,
            tc.tile_pool(name="mintmp", bufs=2) as mpool,
            tc.tile_pool(name="darkp", bufs=1) as dpool,
            tc.tile_pool(name="pad", bufs=1) as ppool,
            tc.tile_pool(name="outp", bufs=5) as opool,
            tc.tile_pool(name="psum", bufs=8, space="PSUM") as pspool,
        ):
            bmat = cpool.tile([128, 3 * SLAB], mybir.dt.bfloat16)
            nc.sync.dma_start(bmat[:], bm[:])

            # per-image dark buffers and pad buffers live across the whole
            # program: dark so neighbouring slabs can read each other's
            # halo columns, pad so the zero edges are set once
            darks = [
                dpool.tile([128, NSLAB * W], mybir.dt.bfloat16, name=f"dark{i}")
                for i in range(IMGS_PER_CORE)
            ]
            padbufs = []
            for i in range(IMGS_PER_CORE):
                for t in range(NSLAB):
                    pb = ppool.tile(
                        [128, PB_W], mybir.dt.float32, name=f"pad_{i}_{t}"
                    )
                    nc.vector.memset(pb[:, 0:PB_LO], 0.0)
                    nc.vector.memset(pb[:, PB_LO + W : PB_W], 0.0)
                    padbufs.append(pb)

            import contextlib

            loop_cm = tc.For_i(0, reps, 1) if reps > 1 else contextlib.nullcontext()
            with loop_cm:
                if mode == "noop":
                    nt = mpool.tile([128, W], mybir.dt.float32, tag="mt")
                    nc.vector.memset(nt[:, 0:8], 0.0)
                else:
                    for i in range(IMGS_PER_CORE):
                        for t in range(NSLAB):
                            _emit_load_min(nc, x, y, darks[i], i, t, chpool,
                                           mpool, mode)
                            if mode != "full":
                                continue
                            if t >= 1:
                                _emit_boxfilter(
                                    nc, y, bmat, scale, darks[i], i, t - 1,
                                    padbufs[i * NSLAB + t - 1], opool, pspool,
                                )
                        if mode == "full":
                            _emit_boxfilter(
                                nc, y, bmat, scale, darks[i], i, NSLAB - 1,
                                padbufs[i * NSLAB + NSLAB - 1], opool, pspool,
                            )
    nc.compile()
    return nc


def _emit_load_min(nc, x, y, dark, i, t, chpool, mpool, mode):
    r0 = t * SLAB
    ch = chpool.tile([128, 3 * W], mybir.dt.float32, tag="ch")
    if mode in ("full", "dma", "dmain", "dmaG"):
        # per-channel DMAs: each reads a 512KB HBM-contiguous block (better
        # row locality than one channel-interleaved transfer)
        for c in range(3):
            nc.sync.dma_start(
                ch[:, c * W : (c + 1) * W], x[i, c, r0 : r0 + SLAB, :]
            )
    if mode == "dma":
        nc.scalar.dma_start(y[i, r0 : r0 + SLAB, :], ch[0:SLAB, 0:W])
    elif mode == "dmaG":
        nc.gpsimd.dma_start(y[i, r0 : r0 + SLAB, :], ch[0:SLAB, 0:W])
    if mode != "full":
        return
    mt = mpool.tile([128, W], mybir.dt.float32, tag="mt")
    nc.vector.tensor_tensor(
        mt[:, :], ch[:, 0:W], ch[:, W : 2 * W], mybir.AluOpType.min
    )
    nc.vector.tensor_tensor(
        dark[:, t * W : (t + 1) * W], mt[:, :], ch[:, 2 * W : 3 * W],
        mybir.AluOpType.min,
    )


def _emit_boxfilter(nc, y, bmat, scale, dark, i, t, pb, opool, pspool):
    r0 = t * SLAB

    # --- 15-tap row sums on TensorE: bands stationary, dark moving; the
    # halo rows come from the neighbour slabs' dark columns via two extra
    # accumulating matmuls ---
    for half in range(2):
        c0 = half * 512
        ps = pspool.tile([128, 512], mybir.dt.float32, tag="ps")
        mms = [(bmat[0:128, 0:SLAB], dark[0:128, t * W + c0 : t * W + c0 + 512])]
        if t > 0:
            mms.append((
                bmat[64:128, SLAB : 2 * SLAB],
                dark[64:128, (t - 1) * W + c0 : (t - 1) * W + c0 + 512],
            ))
        if t < NSLAB - 1:
            mms.append((
                bmat[0:PAD, 2 * SLAB : 3 * SLAB],
                dark[0:PAD, (t + 1) * W + c0 : (t + 1) * W + c0 + 512],
            ))
        for k, (lhsT, rhs) in enumerate(mms):
            nc.tensor.matmul(
                ps[:, :],
                lhsT=lhsT,
                rhs=rhs,
                start=(k == 0),
                stop=(k == len(mms) - 1),
            )
        # --- PSUM -> padbuf with 1/225 scale fused ---
        nc.scalar.activation(
            pb[:, PB_LO + c0 : PB_LO + c0 + 512],
            ps[:, :],
            mybir.ActivationFunctionType.Copy,
            scale=scale,
        )

    # --- 15-tap column sums in one DVE pass: sliding-window recurrence ---
    ot = opool.tile([128, NV], mybir.dt.float32, tag="out")
    nc.vector.tensor_tensor_scan(
        ot[:, :],
        pb[:, PB_LO : PB_LO + NV],
        pb[:, 0:NV],
        0.0,
        mybir.AluOpType.add,
        mybir.AluOpType.subtract,
    )

    # --- store the finished rows (skip the PAD warmup cols); issued from
    # the Activation engine -> second HWDGE ring so the output stream does
    # not serialize behind the input stream ---
    nc.scalar.dma_start(y[i, r0 : r0 + SLAB, :], ot[:, PAD : PAD + W])


def kernel(images, weight):
    global LAST_RESULTS
    images = np.ascontiguousarray(np.asarray(images, dtype=np.float32))
    weight = np.asarray(weight, dtype=np.float64)
    # reference: conv with w = weight/225; weight is uniform (ones), so the
    # whole filter reduces to mean(weight)/225 * boxsum.
    scale = float(weight.mean()) / (KS * KS)

    if scale not in _PROGRAM_CACHE:
        _PROGRAM_CACHE[scale] = _build_program(scale)
    nc = _PROGRAM_CACHE[scale]
    bmat = _build_bmat()
    in_maps = [
        {
            "x": images[c * IMGS_PER_CORE : (c + 1) * IMGS_PER_CORE],
            "bmat": bmat,
        }
        for c in range(N_CORES)
    ]
    res = run_bass_kernel_spmd(nc, in_maps, core_ids=list(range(N_CORES)))
    LAST_RESULTS = res
    out = np.concatenate([r["y"][:, None, :, :] for r in res.results], axis=0)
    return out.astype(np.float32)
